# revision 1
# baseline (speedup 1.0000x reference)
"""GAT-style 3-layer attention graph network on 8 TRN2 NeuronCores.

Math: per layer, alpha[i,j] = adj[i,j]*exp(el[i]+er[j]+ab) / sum_k adj[i,k]*exp(el[i]+er[k]+ab)
The exp(el[i]) factor cancels between numerator and denominator, so with
w[j] = exp(er[j]+ab):
    out[i] = relu( (sum_j adj[i,j]*w[j]*h[j]) / (sum_j adj[i,j]*w[j]) )
i.e. one [N,N]@[N,F+1] matmul per layer against G = [h*w | w], with adj
constant across layers.

Distribution: row-shard adj across the 8 cores (1024 dest rows each). adj is
0/1 so it is exactly representable in fp8_e4m3: the host pre-transposes each
core's row-block into the matmul lhsT tile layout [128, m, k, 128] fp8
(the PE contracts over the partition index, which for the aggregation is
adj's column index), and it stays SBUF-resident (8MB/core) across all 3
layers; the mixed fp8-lhsT x fp16-rhs matmul is exact for 0/1 weights.
Each layer all-gathers the 8192x(F+1) fp16 G matrix (2MB) in two node-halves
so the first gather overlaps the previous layer's aggregation, and the next
layer's G is built inside the per-m epilogue of the current aggregation.
"""
import numpy as np

import concourse.bass as bass
import concourse.mybir as mybir
import concourse.tile as tile
from concourse.masks import make_identity
from concourse.tile_rust import add_dep_helper
from concourse.bass_utils import run_bass_kernel_spmd

F32 = mybir.dt.float32
F16 = mybir.dt.float16  # G storage dtype: 10-bit mantissa
F8 = mybir.dt.float8e4   # adj storage: 0/1 exact in fp8_e4m3, 4x weight-load

N_CORES = 8
N = 8192
NL = N // N_CORES          # 1024 local dest rows per core
NT = NL // 128             # 8 local node tiles
KT = N // 128              # 64 contraction tiles
LEAK = 0.2
H1 = 5                     # node-blocks in the first gather half (of NT=8)


def _split_excess_waits(nc, max_waits=1):
    """This walrus build allows only one sync-wait command per instruction;
    split any instruction carrying more into preceding single-wait nops."""
    n_split = 0
    for fn in nc.m.functions:
        for bb in fn.blocks:
            insts = bb.instructions
            i = 0
            while i < len(insts):
                inst = insts[i]
                si = inst.sync_info
                if si is not None and len(si.on_wait) > max_waits:
                    waits = list(si.on_wait)
                    extra, keep = waits[:-max_waits], waits[-max_waits:]
                    nops = []
                    for j, w in enumerate(extra):
                        nop = mybir.InstNoOp(
                            name=f"{inst.name}-waitsplit-{j}", ins=[], outs=[]
                        )
                        nop.engine = inst.engine
                        nop.sync_info = mybir.SyncInfo(on_wait=[w], on_update=[])
                        nops.append(nop)
                    inst.sync_info = mybir.SyncInfo(
                        on_wait=keep, on_update=list(si.on_update)
                    )
                    insts[i:i] = nops
                    i += len(nops)
                    n_split += 1
                i += 1
    return n_split


def _build_program(ab, for_sim=False):
    """ab: the three attention bias floats (baked in as memset constants)."""
    fhs = [128, 128, 64]  # per-layer linear output width

    nc = bass.Bass(num_devices=N_CORES)

    adj_ext = nc.dram_tensor("adjt", [128, NT, KT, 128], F8, kind="ExternalInput")
    x_ext = nc.dram_tensor("xt_local", [128, NL], F32, kind="ExternalInput")
    # packed params: cols [0:128)=w0t [128:256)=w1t [256:320)=w2t,
    # 320+l = b_l column, 323+l = awr_l column (rows past fh zero-padded)
    par_ext = nc.dram_tensor("params", [128, 326], F32, kind="ExternalInput")
    out_ext = nc.dram_tensor("out", [NL, 64], F32, kind="ExternalOutput")

    # all-gather payload in tiled layout, split in two node-halves per layer
    # so the first gather overlaps the previous aggregation: half h of layer l
    # holds rank blocks [128, 4*(fh+1)] with (p, t, f) = G[c*1024+(4h+t)*128+p, f]
    hblk = [H1, NT - H1]
    ag_ext = [[nc.dram_tensor(f"ag{l}h{h}", [N_CORES * 128, hblk[h] * (fhs[l] + 1)],
                              F16, addr_space="Shared") for h in range(2)]
              for l in range(3)]

    with tile.TileContext(nc) as tc:
        with (
            tc.tile_pool(name="const", bufs=1) as cp,
            tc.tile_pool(name="adjt", bufs=1) as ap_,
            tc.tile_pool(name="slabs", bufs=3) as sp,
            tc.tile_pool(name="gsb", bufs=2) as gp,
            tc.tile_pool(name="misc", bufs=2) as mp,
            tc.tile_pool(name="gloc", bufs=2) as glp,
            tc.tile_pool(name="dram", bufs=3, space="DRAM") as dp,
            tc.tile_pool(name="ptf32", bufs=2, space="PSUM") as ptf32,
            tc.tile_pool(name="plin", bufs=1, space="PSUM") as plin,
            tc.tile_pool(name="per", bufs=1, space="PSUM") as per,
            tc.tile_pool(name="pbig", bufs=4, space="PSUM") as pbig,
        ):
            # ---- constants / params ----
            ident_f32 = cp.tile([128, 128], F32)
            make_identity(nc, ident_f32[:])
            par = cp.tile([128, 326], F32)
            nc.sync.dma_start(out=par[:], in_=par_ext.ap())
            woff = [0, 128, 256]
            wt_sb = [par[:, woff[l]:woff[l] + fhs[l]] for l in range(3)]
            b_sb = [par[0:fhs[l], 320 + l:321 + l] for l in range(3)]
            awr_sb = [par[0:fhs[l], 323 + l:324 + l] for l in range(3)]
            ab_sb = []
            for l in range(3):
                t = cp.tile([128, 1], F32, tag=f"ab{l}")
                nc.gpsimd.memset(t[:], float(ab[l]))
                ab_sb.append(t)

            # ---- x arrives pre-transposed: [fi, node] ----
            curT = sp.tile([128, NL], F32, tag="slab")
            nc.sync.dma_start(out=curT[:], in_=x_ext.ap())

            # ---- adj arrives pre-transposed+tiled from host: [128, m, k, 128]
            # f16; tile (k, m) = adj[m-block rows, k-block cols].T. Load in
            # m-pair chunks so layer-0 m-chains can start after ~1/4 the DMA.
            adjT = ap_.tile([128, NT, KT, 128], F8)
            adjt_insts = []
            for d in range(NT):
                adjt_insts.append(nc.gpsimd.dma_start(
                    out=adjT[:, d, :, :],
                    in_=adj_ext[:, d, :, :],
                ))

            # ---- G-prep helper: one 128-node block of layer l's G ----
            # src_col: [128(fi), 128] column of transposed prev activations
            def prep_block(l, src_col, gl, m):
                fh = fhs[l]
                pl = plin.tile([128, 128], F32, tag="lin")
                nc.tensor.matmul(pl[0:fh, 0:128], wt_sb[l], src_col,
                                 start=True, stop=True)
                hcol = mp.tile([128, 128], F32, tag="hcol")
                nc.scalar.activation(
                    hcol[0:fh, :], pl[0:fh, 0:128],
                    mybir.ActivationFunctionType.Prelu,
                    bias=b_sb[l], scale=1.0, alpha=LEAK,
                )
                pe_ = per.tile([128, 1], F32, tag="er")
                nc.tensor.matmul(pe_[:, 0:1], hcol[0:fh, :], awr_sb[l],
                                 start=True, stop=True)
                ec = mp.tile([128, 1], F32, tag="expc")
                nc.scalar.activation(
                    ec[:], pe_[:, 0:1], mybir.ActivationFunctionType.Exp,
                    bias=ab_sb[l][:], scale=1.0,
                )
                ptg = ptf32.tile([128, 128], F32, tag="ptf")
                nc.tensor.transpose(ptg[:, 0:fh], hcol[0:fh, :],
                                    ident_f32[0:fh, 0:fh])
                nc.vector.tensor_scalar_mul(gl[:, m, 0:fh], ptg[:, 0:fh], ec[:])
                nc.vector.tensor_copy(gl[:, m, fh:fh + 1], ec[:])

            gsb_tiles = {}

            def fire_gather(l, gl, h):
                """All-gather node-half h of layer l's local G block, then
                immediately queue the SBUF reload of that half (so it sits
                before the next gld store in the SP HWDGE FIFO)."""
                fh = fhs[l]
                b0 = 0 if h == 0 else H1
                nb = H1 if h == 0 else NT - H1
                gld = dp.tile([128, nb * (fh + 1)], F16, tag="gld")
                nc.sync.dma_start(out=gld[:], in_=gl[:, b0:b0 + nb, :])
                if for_sim:
                    # stand-in with roughly the real gather's wire time: one
                    # copy per rank block (~4-5us total)
                    for c in range(N_CORES):
                        nc.sync.dma_start(
                            out=ag_ext[l][h][c * 128:(c + 1) * 128, :],
                            in_=gld[:],
                        )
                else:
                    nc.gpsimd.collective_compute(
                        "AllGather", mybir.AluOpType.bypass,
                        replica_groups=[list(range(N_CORES))],
                        ins=[gld.opt()], outs=[ag_ext[l][h].ap().opt()],
                    )
                if h == 0:
                    gsb_new = gp.tile([128, N_CORES, NT, fh + 1], F16, tag="gsb")
                    gsb_tiles[l] = gsb_new
                return nc.sync.dma_start(
                    out=gsb_tiles[l][:, :, b0:b0 + nb, :],
                    in_=ag_ext[l][h].ap().rearrange(
                        "(c p) (t f) -> p c t f", p=128, f=fh + 1
                    ),
                )

            # two-block variant for the layer-0 prologue: halves the
            # serial cross-engine hop count on the startup critical path
            def prep_pair(l, src2, gl, m0):
                fh = fhs[l]
                pl = plin.tile([128, 256], F32, tag="lin")
                nc.tensor.matmul(pl[0:fh, 0:256], wt_sb[l], src2,
                                 start=True, stop=True)
                hcol = mp.tile([128, 256], F32, tag="hcol")
                nc.scalar.activation(
                    hcol[0:fh, :], pl[0:fh, 0:256],
                    mybir.ActivationFunctionType.Prelu,
                    bias=b_sb[l], scale=1.0, alpha=LEAK,
                )
                pe_ = per.tile([128, 2], F32, tag="er")
                for j in range(2):
                    nc.tensor.matmul(pe_[:, j:j + 1],
                                     hcol[0:fh, j * 128:(j + 1) * 128],
                                     awr_sb[l], start=True, stop=True)
                ec = mp.tile([128, 2], F32, tag="expc")
                nc.scalar.activation(
                    ec[:], pe_[:, 0:2], mybir.ActivationFunctionType.Exp,
                    bias=ab_sb[l][:], scale=1.0,
                )
                for j in range(2):
                    ptg = ptf32.tile([128, 128], F32, tag="ptf")
                    nc.tensor.transpose(ptg[:, 0:fh],
                                        hcol[0:fh, j * 128:(j + 1) * 128],
                                        ident_f32[0:fh, 0:fh])
                    nc.vector.tensor_scalar_mul(
                        gl[:, m0 + j, 0:fh], ptg[:, 0:fh], ec[:, j:j + 1])
                    nc.vector.tensor_copy(
                        gl[:, m0 + j, fh:fh + 1], ec[:, j:j + 1])

            # ---- layer 0 G from x (overlaps the adj load) ----
            gl_cur = glp.tile([128, NT, fhs[0] + 1], F16, tag="gloc")
            for m2 in range(0, NT, 2):
                prep_pair(0, curT[:, m2 * 128:(m2 + 2) * 128], gl_cur, m2)
                if m2 == (H1 - 1) // 2 * 2:
                    g0h1 = fire_gather(0, gl_cur, 0)
            g0h2 = fire_gather(0, gl_cur, 1)
            # let layer 0's G reloads jump ahead of the bulk of the adj load:
            # chunks 2+ aren't needed until their m-chains run anyway
            for d in range(2, NT):
                add_dep_helper(adjt_insts[d].ins, g0h1.ins, sync=True,
                               reason="adjt bulk yields to L0 G reload")

            # ---- layers: all-gather G, aggregate, and build next layer's G
            # inside the per-m epilogue so only the collective + G reload sit
            # on the layer boundary ----
            # ---- layers: the epilogue of block m (which also builds the
            # NEXT layer's G block m and fires its gathers) is flushed after
            # the FOLLOWING big-MM chain is emitted -- including across the
            # layer seam -- so the PE never stalls on the small-op chains ----
            def make_epilogue(l, gl_next, ostage):
                fh = fhs[l]

                def epilogue(m, bp):
                    recip = mp.tile([128, 1], F32, tag="recip")
                    nc.vector.reciprocal(recip[:], bp[:, fh:fh + 1])
                    if l < 2:
                        h2 = mp.tile([128, fh], F32, tag="h2")
                        nc.scalar.activation(
                            h2[:], bp[:, 0:fh], mybir.ActivationFunctionType.Relu,
                            bias=0.0, scale=recip[:],
                        )
                        pt = ptf32.tile([128, 128], F32, tag="ptf")
                        nc.tensor.transpose(pt[:, 0:128], h2[:], ident_f32[:])
                        cpcol = mp.tile([128, 128], F32, tag="cpcol")
                        nc.vector.tensor_copy(cpcol[:], pt[:, 0:128])
                        prep_block(l + 1, cpcol[:], gl_next, m)
                        if m == H1 - 1:
                            fire_gather(l + 1, gl_next, 0)
                        elif m == NT - 1:
                            fire_gather(l + 1, gl_next, 1)
                    else:
                        nc.scalar.activation(
                            ostage[:, m, :], bp[:, 0:fh],
                            mybir.ActivationFunctionType.Relu,
                            bias=0.0, scale=recip[:],
                        )
                        if m == NT - 1:
                            nc.sync.dma_start(
                                out=out_ext.ap().rearrange(
                                    "(m p) f -> p m f", p=128),
                                in_=ostage[:],
                            )

                return epilogue

            from collections import deque
            pending = deque()

            def flush(n=None):
                k = len(pending) if n is None else min(n, len(pending))
                for _ in range(k):
                    f, pm, pbp = pending.popleft()
                    f(pm, pbp)

            for l in range(3):
                fh = fhs[l]
                gsb = gsb_tiles[l]
                if l < 2:
                    gl_next = glp.tile([128, NT, fhs[l + 1] + 1], F16, tag="gloc")
                    ostage = None
                else:
                    gl_next = None
                    ostage = mp.tile([128, NT, 64], F32, tag="ostage")
                epi = make_epilogue(l, gl_next, ostage)
                # k = c*NT + t; gather-half 0 covers (k % NT) < NT/2
                ks = [k for k in range(KT) if k % NT < H1] + \
                     [k for k in range(KT) if k % NT >= H1]
                n1 = N_CORES * H1

                def mm_run(bp, m, i0, i1):
                    for i in range(i0, i1):
                        k = ks[i]
                        nc.tensor.matmul(
                            bp[:],
                            adjT[:, m, k, :],
                            gsb[:, k // NT, k % NT, :],
                            start=(i == 0), stop=(i == KT - 1),
                        )

                # seam: emit the k<KT/2 halves of the first two blocks, then
                # flush all pending epilogues (incl. the previous layer's m=7,
                # which fires the G-half-2 gather) before any k>=KT/2 MM
                bp0 = pbig.tile([128, fh + 1], F32, tag="big")
                mm_run(bp0, 0, 0, n1)
                bp1 = pbig.tile([128, fh + 1], F32, tag="big")
                mm_run(bp1, 1, 0, n1)
                flush()
                mm_run(bp0, 0, n1, KT)
                mm_run(bp1, 1, n1, KT)
                pending.append((epi, 0, bp0))
                pending.append((epi, 1, bp1))
                for m in range(2, NT):
                    bp = pbig.tile([128, fh + 1], F32, tag="big")
                    mm_run(bp, m, 0, n1)
                    flush(1)
                    mm_run(bp, m, n1, KT)
                    pending.append((epi, m, bp))
            flush()

    _split_excess_waits(nc)
    return nc


_PROG_CACHE = {}


def _get_program(ab):
    key = tuple(round(a, 9) for a in ab)
    if key not in _PROG_CACHE:
        _PROG_CACHE[key] = _build_program(ab)
    return _PROG_CACHE[key]


def _make_in_maps(inputs):
    """Build the per-core input maps from the full (unsharded) input dict."""
    fhs = [128, 128, 64]
    x = np.asarray(inputs["x"], np.float32)
    adj = np.asarray(inputs["adj"], np.float32)
    in_maps = []
    for c in range(N_CORES):
        import ml_dtypes
        blk = adj[c * NL:(c + 1) * NL, :].astype(ml_dtypes.float8_e4m3)
        # [NL, N] -> [m, q, k, p] -> lhsT tile layout [p, m, k, q]
        adjt = blk.reshape(NT, 128, KT, 128).transpose(3, 0, 2, 1)
        m = {
            "adjt": np.ascontiguousarray(adjt),
            "xt_local": np.ascontiguousarray(x[c * NL:(c + 1) * NL, :].T),
        }
        par = np.zeros((128, 326), np.float32)
        woff = [0, 128, 256]
        for l in range(3):
            W = np.asarray(inputs[f"W{l}"], np.float32)
            b = np.asarray(inputs[f"b{l}"], np.float32)
            aW = np.asarray(inputs[f"aW{l}"], np.float32)
            par[:, woff[l]:woff[l] + fhs[l]] = W.T
            par[:fhs[l], 320 + l] = b.reshape(-1)
            par[:fhs[l], 323 + l] = aW[0, fhs[l]:2 * fhs[l]]
        m["params"] = par
        in_maps.append(m)
    return in_maps


def kernel(x, adj, W0, b0, aW0, ab0, W1, b1, aW1, ab1, W2, b2, aW2, ab2):
    inputs = dict(x=x, adj=adj, W0=W0, b0=b0, aW0=aW0, ab0=ab0,
                  W1=W1, b1=b1, aW1=aW1, ab1=ab1, W2=W2, b2=b2, aW2=aW2, ab2=ab2)
    ab = [float(np.asarray(inputs[f"ab{l}"]).reshape(-1)[0]) for l in range(3)]
    nc = _get_program(ab)
    in_maps = _make_in_maps(inputs)
    res = run_bass_kernel_spmd(nc, in_maps, list(range(N_CORES)))
    out = np.concatenate([res.results[c]["out"] for c in range(N_CORES)], axis=0)
    return out.astype(np.float32)

